# revision 41
# baseline (speedup 1.0000x reference)
"""Trainium2 Bass kernel for GQA attention block (B=1, S=2048, DIM=4096,
32 q heads / 8 kv heads, head_dim 128, RoPE, causal, fused QKV + out proj).

Sharding: tensor-parallel over heads across 8 cores. Core i computes
q heads 4i..4i+3 and kv head i (one full GQA group), plus the wo
contribution of its 512 output columns; host sums the 8 partial outputs.

All matmul operands are fp16 (host-converted): same PE streaming rate as
f32r but LDWEIGHTS is 2x faster and FWL engages, so weight loads hide
under the matmuls; DMA bytes halve. PSUM accumulation stays fp32.
"""
import numpy as np

import concourse.bass as bass
import concourse.mybir as mybir
import concourse.tile as tile
from concourse import bacc
from concourse.bass_utils import run_bass_kernel_spmd
from concourse.masks import make_identity

F32 = mybir.dt.float32
F16 = mybir.dt.float16
AF = mybir.ActivationFunctionType

B, S, DIM = 1, 2048, 4096
N_HEADS, N_KV_HEADS = 32, 8
HD = DIM // N_HEADS              # 128
N_CORES = 8
QH = N_HEADS // N_CORES          # 4 q heads per core
OC = QH * HD + 2 * HD            # 768 per-core qkv output columns
NS = S // 128                    # 16 s-blocks
ND = DIM // 128                  # 32 d-blocks
XSUB = 8                         # d-blocks per x sub-tile in phase 1
NXS = ND // XSUB                 # 4 x sub-tiles per s-block
WSUB = 4                         # d-blocks per w load chunk
STILE = 512                      # s-tile width in phase 2/3
NST = S // STILE                 # 4 s-tiles
NDC = DIM // 512                 # 8 output column chunks
SCALE = 1.0 / float(np.sqrt(HD))
MASK_NEG = -1.0e5


def _build_nc():
    nc = bacc.Bacc("TRN2", target_bir_lowering=False, debug=False)

    # host-pre-tiled inputs (see _prep_in_maps for layouts)
    xt = nc.dram_tensor("xt", [NS, NXS, 128, XSUB, 128], F16,
                        kind="ExternalInput").ap()
    wt = nc.dram_tensor("wt", [128, ND, OC], F16, kind="ExternalInput").ap()
    wot = nc.dram_tensor("wot", [128, NDC, QH, 512], F16,
                         kind="ExternalInput").ap()
    cos5 = nc.dram_tensor("cos5", [S, 5 * 64], F32, kind="ExternalInput").ap()
    sin5 = nc.dram_tensor("sin5", [S, 5 * 64], F32, kind="ExternalInput").ap()
    cmask = nc.dram_tensor("cmask", [128, 4 * STILE], F32, kind="ExternalInput").ap()
    y = nc.dram_tensor("y", [S, DIM], F16, kind="ExternalOutput").ap()

    with tile.TileContext(nc) as tc:
        _emit(tc, nc, xt, wt, wot, cos5, sin5, cmask, y)
    nc.compile()
    return nc


def _emit(tc, nc, xt, wt, wot, cos5, sin5, cmask, y):
    import contextlib

    with contextlib.ExitStack() as ctx:
        # ---------- long-lived tiles ----------
        keep = ctx.enter_context(tc.tile_pool(name="keep", bufs=1))
        # QT_all[:, h, :]: per-head roped Q transposed [d, s]; h=QH is roped K
        QT_all = keep.tile([128, QH + 1, S], F16)
        V_all = keep.tile([128, NS, HD], F16)           # V blocks [t, d]
        identf = keep.tile([128, 128], F32)
        make_identity(nc, identf)
        ident = keep.tile([128, 128], F16)
        nc.vector.tensor_copy(ident, identf)
        ones_f = keep.tile([128, 128], F32)
        nc.vector.memset(ones_f, 1.0)
        ones_r = keep.tile([128, 128], F16)
        nc.vector.tensor_copy(ones_r, ones_f)

        # ---------- phase 1: qkv projection + RoPE + transposes ----------
        with (
            tc.tile_pool(name="p1w", bufs=1) as p1w,
            tc.tile_pool(name="p1x", bufs=2) as p1x,
            tc.tile_pool(name="p1t", bufs=1) as p1t,
            tc.tile_pool(name="p1ps", bufs=1, space="PSUM") as p1ps,
        ):
            keep_tiles = (QT_all, V_all, ident, ones_r)
            # first x sub-tile before the w bulk so PE can start ASAP
            x_first = p1x.tile([128, XSUB, 128], F16, tag="x")
            nc.scalar.dma_start(x_first, xt[0, 0])
            # w chunked so the first matmuls can start after the first chunk
            w_sb = p1w.tile([128, ND, OC], F16)
            wchunks = [(0, 1), (1, 1), (2, 2)] + [
                (c, WSUB) for c in range(WSUB, ND, WSUB)]
            for c0, cn in wchunks:
                nc.sync.dma_start(
                    w_sb[:, c0:c0 + cn, :], wt[:, c0:c0 + cn, :])

            # sb 0-3 run as a group with w-chunk-major matmul order so PE
            # consumption tracks the streaming w arrival; sb 4-15 run one
            # s-block at a time so each block's RoPE (DVE) hides under the
            # next block's matmuls. Transposes trail by one s-block so they
            # never wait on the rope that was just emitted.
            pend_t = []

            def flush_transposes():
                while pend_t:
                    pend_t.pop(0)()

            GRP = 4
            group0 = list(range(GRP))
            ps_qs = {}
            ps_kvs = {}
            x_tiles = {}
            for sb in group0:
                ps_qs[sb] = p1ps.tile([128, 512], F32, tag=f"psq{sb % GRP}", name=f"psq{sb}")
                ps_kvs[sb] = p1ps.tile([128, 256], F32, tag=f"pskv{sb % GRP}", name=f"pskv{sb}")
            for sb in group0:
                if sb == 0:
                    x_tiles[sb] = x_first
                else:
                    x_tiles[sb] = p1x.tile(
                        [128, XSUB, 128], F16, tag=f"x{sb % GRP}",
                        name=f"x{sb}_0")
                    eng = nc.scalar if sb % 2 == 0 else nc.gpsimd
                    eng.dma_start(x_tiles[sb], xt[sb, 0])
            for xs in range(NXS):
                nxt_tiles = {}
                if xs < NXS - 1:
                    # issue next pass's x tiles now: a full pass of
                    # lookahead so the pass boundary never waits on DMA
                    for sb in group0:
                        nxt_tiles[sb] = p1x.tile(
                            [128, XSUB, 128], F16, tag=f"x{sb % GRP}",
                            name=f"x{sb}_{xs + 1}")
                        eng = nc.scalar if sb % 2 == 0 else nc.gpsimd
                        eng.dma_start(nxt_tiles[sb], xt[sb, xs + 1])
                for sb in group0:
                    x_sb = x_tiles[sb]
                    for dbi in range(XSUB):
                        db = XSUB * xs + dbi
                        nc.tensor.matmul(
                            ps_qs[sb], lhsT=x_sb[:, dbi, :],
                            rhs=w_sb[:, db, 0:512],
                            start=(db == 0), stop=(db == ND - 1),
                        )
                        nc.tensor.matmul(
                            ps_kvs[sb], lhsT=x_sb[:, dbi, :],
                            rhs=w_sb[:, db, 512:768],
                            start=(db == 0), stop=(db == ND - 1),
                        )
                    if xs == NXS - 1:
                        flush_transposes()
                        _rope_and_transpose(
                            tc, nc, p1t, p1ps, cos5, sin5, sb,
                            ps_qs[sb], ps_kvs[sb], QT_all, V_all, ident,
                            pend_t)
                x_tiles = nxt_tiles

            for sb in range(GRP, NS):
                ps_q = p1ps.tile([128, 512], F32, tag=f"psq{sb % GRP}", name=f"psq{sb}")
                ps_kv = p1ps.tile([128, 256], F32, tag=f"pskv{sb % GRP}", name=f"pskv{sb}")
                for xs in range(NXS):
                    x_sb = p1x.tile([128, XSUB, 128], F16, tag=f"xs{xs}",
                                    name=f"x{sb}_{xs}")
                    # gpsimd only: on the scalar queue these loads sit
                    # behind the previous block's QT/V copies (head-of-line
                    # blocking) and arrive late for the next block's matmuls
                    nc.gpsimd.dma_start(x_sb, xt[sb, xs])
                    for dbi in range(XSUB):
                        db = XSUB * xs + dbi
                        nc.tensor.matmul(
                            ps_q, lhsT=x_sb[:, dbi, :],
                            rhs=w_sb[:, db, 0:512],
                            start=(db == 0), stop=(db == ND - 1),
                        )
                        nc.tensor.matmul(
                            ps_kv, lhsT=x_sb[:, dbi, :],
                            rhs=w_sb[:, db, 512:768],
                            start=(db == 0), stop=(db == ND - 1),
                        )
                flush_transposes()
                _rope_and_transpose(
                    tc, nc, p1t, p1ps, cos5, sin5, sb,
                    ps_q, ps_kv, QT_all, V_all, ident, pend_t)
            flush_transposes()

        _emit_attn(tc, nc, ctx, keep_tiles, wot, y, cmask)


def _rope_and_transpose(tc, nc, p1t, p1ps, cos5, sin5, sb, ps_q, ps_kv,
                        QT_all, V_all, ident, pend_t):
    # RoPE (q: 4 heads = 512 cols; k: 128 cols)
    cos_t = p1t.tile([128, 320], F32, tag="cos")
    sin_t = p1t.tile([128, 320], F32, tag="sin")
    nc.gpsimd.dma_start(cos_t, cos5[128 * sb:128 * (sb + 1), :])
    nc.gpsimd.dma_start(sin_t, sin5[128 * sb:128 * (sb + 1), :])

    qk_roped = p1t.tile([128, 640], F16, tag=f"qkr{sb % 2}")
    for part, ps_src, wid in (("q", ps_q, 512), ("k", ps_kv, 128)):
        nf = wid // 2
        off = 0 if part == "q" else 512
        pe = ps_src[:, 0:wid:2]
        po = ps_src[:, 1:wid:2]
        c = cos_t[:, 0:nf]
        sn = sin_t[:, 0:nf]
        t1 = p1t.tile([128, 256], F32, tag="t1")
        t2 = p1t.tile([128, 256], F32, tag="t2")
        nc.vector.tensor_mul(t1[:, 0:nf], pe, c)
        nc.vector.tensor_mul(t2[:, 0:nf], po, sn)
        nc.vector.tensor_sub(
            qk_roped[:, off + 0:off + wid:2], t1[:, 0:nf], t2[:, 0:nf])
        t3 = p1t.tile([128, 256], F32, tag="t3")
        t4 = p1t.tile([128, 256], F32, tag="t4")
        nc.vector.tensor_mul(t3[:, 0:nf], pe, sn)
        nc.vector.tensor_mul(t4[:, 0:nf], po, c)
        nc.vector.tensor_add(
            qk_roped[:, off + 1:off + wid:2], t3[:, 0:nf], t4[:, 0:nf])

    # V block: natural [t, d] (scalar engine: DVE is busy with rope math)
    nc.scalar.copy(V_all[:, sb, :], ps_kv[:, 128:256])

    # transpose roped q/k head-slices into QT_all (deferred one s-block)
    def emit(sb=sb, qk_roped=qk_roped):
        for h in range(QH + 1):
            # borrow kv accumulator slots (pool-tag reuse; tile's WAR
            # tracking orders this after the rope/V reads)
            tag = f"psq{sb % 4}" if h % 2 == 0 else f"pskv{sb % 4}"
            ps_t = p1ps.tile([128, 128], F16, tag=tag,
                             name=f"pst{sb}_{h}")
            nc.tensor.transpose(ps_t, qk_roped[:, 128 * h:128 * (h + 1)], ident)
            nc.scalar.copy(
                QT_all[:, h, 128 * sb:128 * (sb + 1)], ps_t)
    pend_t.append(emit)


def _emit_attn(tc, nc, ctx, keep_tiles, wot, y, cmask):
    (QT_all, V_all, ident, ones_r) = keep_tiles
    # ---------- phase 2: attention per head ----------
    p2keep = ctx.enter_context(tc.tile_pool(name="p2keep", bufs=1))
    OT_all = p2keep.tile([128, QH, S], F16)         # attn out transposed
    cmask_sb = p2keep.tile([128, 4, STILE], F32)
    nc.gpsimd.dma_start(cmask_sb, cmask.rearrange("p (k s) -> p k s", k=4))

    p3w = ctx.enter_context(tc.tile_pool(name="p3w", bufs=1))
    wo_full = p3w.tile([128, NDC, QH, 512], F16)
    for dc in range(NDC):
        nc.sync.dma_start(wo_full[:, dc], wot[:, dc])
    with (
        tc.tile_pool(name="p2et", bufs=1) as p2et,
        tc.tile_pool(name="p2t", bufs=4) as p2t,
        tc.tile_pool(name="p2ps", bufs=3, space="PSUM") as p2ps,
        tc.tile_pool(name="p2acc", bufs=1, space="PSUM") as p2acc,
    ):
        for st in range(NST):
            for h in range(QH):
                nj = 4 * st + 4          # number of t-blocks
                npair = nj // 2
                s0 = STILE * st
                ET = p2et.tile([128, NS, STILE], F16, tag="et")
                ETP = p2et.tile([128, NS // 2, STILE], F16, tag="etp")
                ps_av = p2acc.tile([128, STILE], F32, tag="av")
                ps_den = p2acc.tile([128, STILE], F32, tag="den")

                def joff(j):
                    # exact causal trim, column-aligned: t-block j only
                    # attends s >= 128*j, i.e. columns [off, 512) of the
                    # s-tile; the first 128 of those are the triangle.
                    k = j - (nj - 4)
                    return 128 * k if k > 0 else 0

                # exp groups: always pairs of t-blocks (fewest Scalar
                # instructions); the batched exp writes a little garbage
                # into the trimmed rows' masked heads, zeroed right after
                egroups = [(j, j + 1) for j in range(0, nj, 2)]

                def emit_scores(g):
                    js = egroups[g]
                    ps_pr = p2ps.tile([128, 2, STILE], F32, tag="stp",
                                      name=f"sp{h}_{st}_{g}")
                    for jj, j in enumerate(js):
                        off = joff(j)
                        nc.tensor.matmul(
                            ps_pr[:, jj, off:STILE],
                            lhsT=QT_all[:, QH, 128 * j:128 * (j + 1)],
                            rhs=QT_all[:, h, s0 + off:s0 + STILE],
                            start=True, stop=True, skip_group_check=True,
                        )
                        k = j - (nj - 4)
                        if k >= 0:
                            # mask only the 128-wide triangle slab
                            nc.vector.tensor_add(
                                ps_pr[:, jj, 128 * k:128 * (k + 1)],
                                ps_pr[:, jj, 128 * k:128 * (k + 1)],
                                cmask_sb[:, 0, 0:128])
                    goff = joff(js[0])
                    nc.scalar.activation(
                        ET[:, js[0]:js[1] + 1, goff:STILE],
                        ps_pr[:, 0:2, goff:STILE],
                        AF.Exp, scale=SCALE)
                    for j in js:
                        if joff(j):
                            # zero the fully-masked head of the ET row so
                            # the pair-sum for the denominator stays exact
                            nc.gpsimd.memset(ET[:, j, 0:joff(j)], 0.0)
                    # pair-sum for the denominator (fp16 DVE, 2x rate)
                    nc.vector.tensor_add(
                        ETP[:, js[0] // 2, :], ET[:, js[0], :],
                        ET[:, js[1], :])

                def emit_av(g):
                    for j in egroups[g]:
                        off = joff(j)
                        nc.tensor.matmul(
                            ps_av[:, off:STILE], lhsT=V_all[:, j, :],
                            rhs=ET[:, j, off:STILE],
                            start=(j == 0), stop=(j == nj - 1),
                            skip_group_check=True,
                        )

                # software pipeline, depth 2: scores run two groups ahead
                # of AV so each group's exp (Scalar) hides under PE work
                ng = len(egroups)
                for g in range(ng + 2):
                    if g < ng:
                        emit_scores(g)
                    if g >= 2:
                        emit_av(g - 2)
                # denominator on the pair-summed ET: half the PE rows
                for p in range(npair):
                    nc.tensor.matmul(
                        ps_den, lhsT=ones_r, rhs=ETP[:, p, :],
                        start=(p == 0), stop=(p == npair - 1),
                    )

                den_r = p2t.tile([128, STILE], F32, tag="denr")
                nc.vector.reciprocal_approx_fast(den_r, ps_den)
                nc.vector.tensor_mul(OT_all[:, h, s0:s0 + STILE], ps_av, den_r)

    # ---------- phase 3: output projection ----------
    # sb-outer so the y writes drain evenly instead of backing up behind
    # the final weight chunk; wo is fully resident (loaded during p1/p2)
    with (
        tc.tile_pool(name="p3y", bufs=6) as p3y,
        tc.tile_pool(name="p3ps", bufs=6, space="PSUM") as p3ps,
    ):
        for sb in range(NS):
            for dc in range(NDC):
                ps_y = p3ps.tile([128, 512], F32, tag="psy")
                for ob in range(QH):
                    nc.tensor.matmul(
                        ps_y,
                        lhsT=OT_all[:, ob, 128 * sb:128 * (sb + 1)],
                        rhs=wo_full[:, dc, ob, :],
                        start=(ob == 0), stop=(ob == QH - 1),
                    )
                if dc % 2 == 0:
                    y2 = p3y.tile([128, 2, 512], F16, tag=f"y{(dc // 2) % 2}",
                                  name=f"y{sb}_{dc}")
                if (sb + dc) % 2 == 0:
                    nc.vector.tensor_copy(y2[:, dc % 2, :], ps_y)
                else:
                    nc.scalar.copy(y2[:, dc % 2, :], ps_y)
                if dc % 2 == 1:
                    # one 2 KB/partition write per dc-pair on HWDGE queues
                    # only: the gpsimd (SWDGE) queue drains ~9us at exit
                    eng = nc.scalar if dc % 4 == 1 else nc.sync
                    eng.dma_start(
                        y[128 * sb:128 * (sb + 1),
                          1024 * (dc // 2):1024 * (dc // 2 + 1)], y2)


_NC_CACHE = None


def _get_nc():
    global _NC_CACHE
    if _NC_CACHE is None:
        _NC_CACHE = _build_nc()
    return _NC_CACHE


def _prep_in_maps(x, freqs_cos, freqs_sin, wqkv, wo):
    xT = x.reshape(S, DIM).T                                   # [DIM, S]
    # xt[sb, xs, p, n, s] = xT[128*(XSUB*xs+n)+p, 128*sb+s]
    xt = np.ascontiguousarray(
        xT.reshape(NXS, XSUB, 128, NS, 128).transpose(3, 0, 2, 1, 4)
    ).astype(np.float16)
    cos5 = np.ascontiguousarray(np.tile(freqs_cos, (1, 5)))    # [S, 320]
    sin5 = np.ascontiguousarray(np.tile(freqs_sin, (1, 5)))

    # causal masks for the 4 diagonal 128-blocks of a 512-wide s-tile
    tl = np.arange(128)[:, None]
    sl = np.arange(STILE)[None, :]
    cm = np.zeros((128, 4, STILE), np.float32)
    for k in range(4):
        cm[:, k, :] = np.where(sl >= 128 * k + tl, 0.0, MASK_NEG)
    cm = np.ascontiguousarray(cm.reshape(128, 4 * STILE))

    in_maps = []
    for i in range(N_CORES):
        wq = wqkv[QH * HD * i: QH * HD * (i + 1)]               # [512, DIM]
        wk = wqkv[N_HEADS * HD + HD * i: N_HEADS * HD + HD * (i + 1)]
        wv = wqkv[N_HEADS * HD + N_KV_HEADS * HD + HD * i:
                  N_HEADS * HD + N_KV_HEADS * HD + HD * (i + 1)]
        wT = np.concatenate([wq, wk, wv], axis=0).T             # [DIM, 768]
        # wt[p, db, o] = wT[128*db+p, o]
        wt = np.ascontiguousarray(
            wT.reshape(ND, 128, OC).transpose(1, 0, 2)).astype(np.float16)
        woT = wo[:, QH * HD * i: QH * HD * (i + 1)].T           # [512, DIM]
        # wot[p, dc, ob, j] = woT[128*ob+p, 512*dc+j]
        wot = np.ascontiguousarray(
            woT.reshape(QH, 128, NDC, 512).transpose(1, 2, 0, 3)
        ).astype(np.float16)
        in_maps.append({
            "xt": xt, "wt": wt, "wot": wot,
            "cos5": cos5, "sin5": sin5, "cmask": cm,
        })
    return in_maps


def kernel(x, freqs_cos, freqs_sin, mask, wqkv, wo, _want_trace=False):
    x = np.asarray(x, np.float32)
    freqs_cos = np.asarray(freqs_cos, np.float32)
    freqs_sin = np.asarray(freqs_sin, np.float32)
    wqkv = np.asarray(wqkv, np.float32)
    wo = np.asarray(wo, np.float32)

    nc = _get_nc()
    in_maps = _prep_in_maps(x, freqs_cos, freqs_sin, wqkv, wo)
    res = run_bass_kernel_spmd(
        nc, in_maps, core_ids=list(range(N_CORES)), trace=_want_trace,
    )
    out = np.zeros((S, DIM), np.float64)
    for r in res.results:
        out += r["y"].astype(np.float64)
    if _want_trace:
        kernel._last_results = res
    return out.astype(np.float32).reshape(B, S, DIM)


# revision 42
# speedup vs baseline: 1.0181x; 1.0181x over previous
"""Trainium2 Bass kernel for GQA attention block (B=1, S=2048, DIM=4096,
32 q heads / 8 kv heads, head_dim 128, RoPE, causal, fused QKV + out proj).

Sharding: tensor-parallel over heads across 8 cores. Core i computes
q heads 4i..4i+3 and kv head i (one full GQA group), plus the wo
contribution of its 512 output columns; host sums the 8 partial outputs.

All matmul operands are fp16 (host-converted): same PE streaming rate as
f32r but LDWEIGHTS is 2x faster and FWL engages, so weight loads hide
under the matmuls; DMA bytes halve. PSUM accumulation stays fp32.
"""
import numpy as np

import concourse.bass as bass
import concourse.mybir as mybir
import concourse.tile as tile
from concourse import bacc
from concourse.bass_utils import run_bass_kernel_spmd
from concourse.masks import make_identity

F32 = mybir.dt.float32
F16 = mybir.dt.float16
AF = mybir.ActivationFunctionType

B, S, DIM = 1, 2048, 4096
N_HEADS, N_KV_HEADS = 32, 8
HD = DIM // N_HEADS              # 128
N_CORES = 8
QH = N_HEADS // N_CORES          # 4 q heads per core
OC = QH * HD + 2 * HD            # 768 per-core qkv output columns
NS = S // 128                    # 16 s-blocks
ND = DIM // 128                  # 32 d-blocks
XSUB = 8                         # d-blocks per x sub-tile in phase 1
NXS = ND // XSUB                 # 4 x sub-tiles per s-block
WSUB = 4                         # d-blocks per w load chunk
STILE = 512                      # s-tile width in phase 2/3
NST = S // STILE                 # 4 s-tiles
NDC = DIM // 512                 # 8 output column chunks
SCALE = 1.0 / float(np.sqrt(HD))
MASK_NEG = -1.0e5


def _build_nc():
    nc = bacc.Bacc("TRN2", target_bir_lowering=False, debug=False)

    # host-pre-tiled inputs (see _prep_in_maps for layouts)
    xt = nc.dram_tensor("xt", [NS, NXS, 128, XSUB, 128], F16,
                        kind="ExternalInput").ap()
    wt = nc.dram_tensor("wt", [128, ND, OC], F16, kind="ExternalInput").ap()
    wot = nc.dram_tensor("wot", [128, NDC, QH, 512], F16,
                         kind="ExternalInput").ap()
    cos5 = nc.dram_tensor("cos5", [S, 5 * 64], F32, kind="ExternalInput").ap()
    sin5 = nc.dram_tensor("sin5", [S, 5 * 64], F32, kind="ExternalInput").ap()
    cmask = nc.dram_tensor("cmask", [128, 4 * STILE], F32, kind="ExternalInput").ap()
    y = nc.dram_tensor("y", [S, DIM], F16, kind="ExternalOutput").ap()

    with tile.TileContext(nc) as tc:
        _emit(tc, nc, xt, wt, wot, cos5, sin5, cmask, y)
    nc.compile()
    return nc


def _emit(tc, nc, xt, wt, wot, cos5, sin5, cmask, y):
    import contextlib

    with contextlib.ExitStack() as ctx:
        # ---------- long-lived tiles ----------
        keep = ctx.enter_context(tc.tile_pool(name="keep", bufs=1))
        # QT_all[:, h, :]: per-head roped Q transposed [d, s]; h=QH is roped K
        QT_all = keep.tile([128, QH + 1, S], F16)
        V_all = keep.tile([128, NS, HD], F16)           # V blocks [t, d]
        identf = keep.tile([128, 128], F32)
        make_identity(nc, identf)
        ident = keep.tile([128, 128], F16)
        nc.vector.tensor_copy(ident, identf)
        ones_f = keep.tile([128, 128], F32)
        nc.vector.memset(ones_f, 1.0)
        ones_r = keep.tile([128, 128], F16)
        nc.vector.tensor_copy(ones_r, ones_f)

        # ---------- phase 1: qkv projection + RoPE + transposes ----------
        with (
            tc.tile_pool(name="p1w", bufs=1) as p1w,
            tc.tile_pool(name="p1x", bufs=2) as p1x,
            tc.tile_pool(name="p1t", bufs=1) as p1t,
            tc.tile_pool(name="p1ps", bufs=1, space="PSUM") as p1ps,
        ):
            keep_tiles = (QT_all, V_all, ident, ones_r)
            # first x sub-tile before the w bulk so PE can start ASAP
            x_first = p1x.tile([128, XSUB, 128], F16, tag="x")
            nc.scalar.dma_start(x_first, xt[0, 0])
            # w chunked so the first matmuls can start after the first chunk
            w_sb = p1w.tile([128, ND, OC], F16)
            wchunks = [(0, 1), (1, 1), (2, 2)] + [
                (c, WSUB) for c in range(WSUB, ND, WSUB)]
            for c0, cn in wchunks:
                nc.sync.dma_start(
                    w_sb[:, c0:c0 + cn, :], wt[:, c0:c0 + cn, :])

            # sb 0-3 run as a group with w-chunk-major matmul order so PE
            # consumption tracks the streaming w arrival; sb 4-15 run one
            # s-block at a time so each block's RoPE (DVE) hides under the
            # next block's matmuls. Transposes trail by one s-block so they
            # never wait on the rope that was just emitted.
            pend_t = []

            def flush_transposes():
                while pend_t:
                    pend_t.pop(0)()

            GRP = 4
            group0 = list(range(GRP))
            ps_qs = {}
            ps_kvs = {}
            x_tiles = {}
            for sb in group0:
                ps_qs[sb] = p1ps.tile([128, 512], F32, tag=f"psq{sb % GRP}", name=f"psq{sb}")
                ps_kvs[sb] = p1ps.tile([128, 256], F32, tag=f"pskv{sb % GRP}", name=f"pskv{sb}")
            for sb in group0:
                if sb == 0:
                    x_tiles[sb] = x_first
                else:
                    x_tiles[sb] = p1x.tile(
                        [128, XSUB, 128], F16, tag=f"x{sb % GRP}",
                        name=f"x{sb}_0")
                    eng = nc.scalar if sb % 2 == 0 else nc.gpsimd
                    eng.dma_start(x_tiles[sb], xt[sb, 0])
            for xs in range(NXS):
                nxt_tiles = {}
                if xs < NXS - 1:
                    # issue next pass's x tiles now: a full pass of
                    # lookahead so the pass boundary never waits on DMA
                    for sb in group0:
                        nxt_tiles[sb] = p1x.tile(
                            [128, XSUB, 128], F16, tag=f"x{sb % GRP}",
                            name=f"x{sb}_{xs + 1}")
                        eng = nc.scalar if sb % 2 == 0 else nc.gpsimd
                        eng.dma_start(nxt_tiles[sb], xt[sb, xs + 1])
                for sb in group0:
                    x_sb = x_tiles[sb]
                    for dbi in range(XSUB):
                        db = XSUB * xs + dbi
                        nc.tensor.matmul(
                            ps_qs[sb], lhsT=x_sb[:, dbi, :],
                            rhs=w_sb[:, db, 0:512],
                            start=(db == 0), stop=(db == ND - 1),
                        )
                        nc.tensor.matmul(
                            ps_kvs[sb], lhsT=x_sb[:, dbi, :],
                            rhs=w_sb[:, db, 512:768],
                            start=(db == 0), stop=(db == ND - 1),
                        )
                    if xs == NXS - 1:
                        flush_transposes()
                        _rope_and_transpose(
                            tc, nc, p1t, p1ps, cos5, sin5, sb,
                            ps_qs[sb], ps_kvs[sb], QT_all, V_all, ident,
                            pend_t)
                x_tiles = nxt_tiles

            for sb in range(GRP, NS):
                ps_q = p1ps.tile([128, 512], F32, tag=f"psq{sb % GRP}", name=f"psq{sb}")
                ps_kv = p1ps.tile([128, 256], F32, tag=f"pskv{sb % GRP}", name=f"pskv{sb}")
                for xs in range(NXS):
                    x_sb = p1x.tile([128, XSUB, 128], F16, tag=f"xs{xs}",
                                    name=f"x{sb}_{xs}")
                    eng = nc.scalar if xs % 2 == 0 else nc.gpsimd
                    eng.dma_start(x_sb, xt[sb, xs])
                    for dbi in range(XSUB):
                        db = XSUB * xs + dbi
                        nc.tensor.matmul(
                            ps_q, lhsT=x_sb[:, dbi, :],
                            rhs=w_sb[:, db, 0:512],
                            start=(db == 0), stop=(db == ND - 1),
                        )
                        nc.tensor.matmul(
                            ps_kv, lhsT=x_sb[:, dbi, :],
                            rhs=w_sb[:, db, 512:768],
                            start=(db == 0), stop=(db == ND - 1),
                        )
                flush_transposes()
                _rope_and_transpose(
                    tc, nc, p1t, p1ps, cos5, sin5, sb,
                    ps_q, ps_kv, QT_all, V_all, ident, pend_t)
            flush_transposes()

        _emit_attn(tc, nc, ctx, keep_tiles, wot, y, cmask)


def _rope_and_transpose(tc, nc, p1t, p1ps, cos5, sin5, sb, ps_q, ps_kv,
                        QT_all, V_all, ident, pend_t):
    # RoPE (q: 4 heads = 512 cols; k: 128 cols)
    cos_t = p1t.tile([128, 320], F32, tag="cos")
    sin_t = p1t.tile([128, 320], F32, tag="sin")
    nc.gpsimd.dma_start(cos_t, cos5[128 * sb:128 * (sb + 1), :])
    nc.gpsimd.dma_start(sin_t, sin5[128 * sb:128 * (sb + 1), :])

    qk_roped = p1t.tile([128, 640], F16, tag=f"qkr{sb % 2}")
    for part, ps_src, wid in (("q", ps_q, 512), ("k", ps_kv, 128)):
        nf = wid // 2
        off = 0 if part == "q" else 512
        pe = ps_src[:, 0:wid:2]
        po = ps_src[:, 1:wid:2]
        c = cos_t[:, 0:nf]
        sn = sin_t[:, 0:nf]
        t1 = p1t.tile([128, 256], F32, tag="t1")
        t2 = p1t.tile([128, 256], F32, tag="t2")
        nc.vector.tensor_mul(t1[:, 0:nf], pe, c)
        nc.vector.tensor_mul(t2[:, 0:nf], po, sn)
        nc.vector.tensor_sub(
            qk_roped[:, off + 0:off + wid:2], t1[:, 0:nf], t2[:, 0:nf])
        t3 = p1t.tile([128, 256], F32, tag="t3")
        t4 = p1t.tile([128, 256], F32, tag="t4")
        nc.vector.tensor_mul(t3[:, 0:nf], pe, sn)
        nc.vector.tensor_mul(t4[:, 0:nf], po, c)
        nc.vector.tensor_add(
            qk_roped[:, off + 1:off + wid:2], t3[:, 0:nf], t4[:, 0:nf])

    # V block: natural [t, d] (scalar engine: DVE is busy with rope math)
    nc.scalar.copy(V_all[:, sb, :], ps_kv[:, 128:256])

    # transpose roped q/k head-slices into QT_all (deferred one s-block)
    def emit(sb=sb, qk_roped=qk_roped):
        for h in range(QH + 1):
            # borrow kv accumulator slots (pool-tag reuse; tile's WAR
            # tracking orders this after the rope/V reads)
            tag = f"psq{sb % 4}" if h % 2 == 0 else f"pskv{sb % 4}"
            ps_t = p1ps.tile([128, 128], F16, tag=tag,
                             name=f"pst{sb}_{h}")
            nc.tensor.transpose(ps_t, qk_roped[:, 128 * h:128 * (h + 1)], ident)
            nc.scalar.copy(
                QT_all[:, h, 128 * sb:128 * (sb + 1)], ps_t)
    pend_t.append(emit)


def _emit_attn(tc, nc, ctx, keep_tiles, wot, y, cmask):
    (QT_all, V_all, ident, ones_r) = keep_tiles
    # ---------- phase 2: attention per head ----------
    p2keep = ctx.enter_context(tc.tile_pool(name="p2keep", bufs=1))
    OT_all = p2keep.tile([128, QH, S], F16)         # attn out transposed
    cmask_sb = p2keep.tile([128, 4, STILE], F32)
    nc.gpsimd.dma_start(cmask_sb, cmask.rearrange("p (k s) -> p k s", k=4))

    p3w = ctx.enter_context(tc.tile_pool(name="p3w", bufs=1))
    wo_full = p3w.tile([128, NDC, QH, 512], F16)
    for dc in range(NDC):
        nc.sync.dma_start(wo_full[:, dc], wot[:, dc])
    with (
        tc.tile_pool(name="p2et", bufs=1) as p2et,
        tc.tile_pool(name="p2t", bufs=4) as p2t,
        tc.tile_pool(name="p2ps", bufs=3, space="PSUM") as p2ps,
        tc.tile_pool(name="p2acc", bufs=1, space="PSUM") as p2acc,
    ):
        for st in range(NST):
            for h in range(QH):
                nj = 4 * st + 4          # number of t-blocks
                npair = nj // 2
                s0 = STILE * st
                ET = p2et.tile([128, NS, STILE], F16, tag="et")
                ETP = p2et.tile([128, NS // 2, STILE], F16, tag="etp")
                ps_av = p2acc.tile([128, STILE], F32, tag="av")
                ps_den = p2acc.tile([128, STILE], F32, tag="den")

                def joff(j):
                    # exact causal trim, column-aligned: t-block j only
                    # attends s >= 128*j, i.e. columns [off, 512) of the
                    # s-tile; the first 128 of those are the triangle.
                    k = j - (nj - 4)
                    return 128 * k if k > 0 else 0

                # exp groups: always pairs of t-blocks (fewest Scalar
                # instructions); the batched exp writes a little garbage
                # into the trimmed rows' masked heads, zeroed right after
                egroups = [(j, j + 1) for j in range(0, nj, 2)]

                def emit_scores(g):
                    js = egroups[g]
                    ps_pr = p2ps.tile([128, 2, STILE], F32, tag="stp",
                                      name=f"sp{h}_{st}_{g}")
                    for jj, j in enumerate(js):
                        off = joff(j)
                        nc.tensor.matmul(
                            ps_pr[:, jj, off:STILE],
                            lhsT=QT_all[:, QH, 128 * j:128 * (j + 1)],
                            rhs=QT_all[:, h, s0 + off:s0 + STILE],
                            start=True, stop=True, skip_group_check=True,
                        )
                        k = j - (nj - 4)
                        if k >= 0:
                            # mask only the 128-wide triangle slab
                            nc.vector.tensor_add(
                                ps_pr[:, jj, 128 * k:128 * (k + 1)],
                                ps_pr[:, jj, 128 * k:128 * (k + 1)],
                                cmask_sb[:, 0, 0:128])
                    goff = joff(js[0])
                    nc.scalar.activation(
                        ET[:, js[0]:js[1] + 1, goff:STILE],
                        ps_pr[:, 0:2, goff:STILE],
                        AF.Exp, scale=SCALE)
                    for j in js:
                        if joff(j):
                            # zero the fully-masked head of the ET row so
                            # the pair-sum for the denominator stays exact
                            nc.gpsimd.memset(ET[:, j, 0:joff(j)], 0.0)
                    # pair-sum for the denominator (fp16 DVE, 2x rate)
                    nc.vector.tensor_add(
                        ETP[:, js[0] // 2, :], ET[:, js[0], :],
                        ET[:, js[1], :])

                def emit_av(g):
                    for j in egroups[g]:
                        off = joff(j)
                        nc.tensor.matmul(
                            ps_av[:, off:STILE], lhsT=V_all[:, j, :],
                            rhs=ET[:, j, off:STILE],
                            start=(j == 0), stop=(j == nj - 1),
                            skip_group_check=True,
                        )

                # software pipeline, depth 2: scores run two groups ahead
                # of AV so each group's exp (Scalar) hides under PE work
                ng = len(egroups)
                for g in range(ng + 2):
                    if g < ng:
                        emit_scores(g)
                    if g >= 2:
                        emit_av(g - 2)
                # denominator on the pair-summed ET: half the PE rows
                for p in range(npair):
                    nc.tensor.matmul(
                        ps_den, lhsT=ones_r, rhs=ETP[:, p, :],
                        start=(p == 0), stop=(p == npair - 1),
                    )

                den_r = p2t.tile([128, STILE], F32, tag="denr")
                nc.vector.reciprocal_approx_fast(den_r, ps_den)
                nc.vector.tensor_mul(OT_all[:, h, s0:s0 + STILE], ps_av, den_r)

    # ---------- phase 3: output projection ----------
    # sb-outer so the y writes drain evenly instead of backing up behind
    # the final weight chunk; wo is fully resident (loaded during p1/p2)
    with (
        tc.tile_pool(name="p3y", bufs=6) as p3y,
        tc.tile_pool(name="p3ps", bufs=6, space="PSUM") as p3ps,
    ):
        for sb in range(NS):
            for dc in range(NDC):
                ps_y = p3ps.tile([128, 512], F32, tag="psy")
                for ob in range(QH):
                    nc.tensor.matmul(
                        ps_y,
                        lhsT=OT_all[:, ob, 128 * sb:128 * (sb + 1)],
                        rhs=wo_full[:, dc, ob, :],
                        start=(ob == 0), stop=(ob == QH - 1),
                    )
                if dc % 2 == 0:
                    y2 = p3y.tile([128, 2, 512], F16, tag=f"y{(dc // 2) % 2}",
                                  name=f"y{sb}_{dc}")
                if (sb + dc) % 2 == 0:
                    nc.vector.tensor_copy(y2[:, dc % 2, :], ps_y)
                else:
                    nc.scalar.copy(y2[:, dc % 2, :], ps_y)
                if dc % 2 == 1:
                    # one 2 KB/partition write per dc-pair on HWDGE queues
                    # only: the gpsimd (SWDGE) queue drains ~9us at exit
                    eng = nc.scalar if dc % 4 == 1 else nc.sync
                    eng.dma_start(
                        y[128 * sb:128 * (sb + 1),
                          1024 * (dc // 2):1024 * (dc // 2 + 1)], y2)


_NC_CACHE = None


def _get_nc():
    global _NC_CACHE
    if _NC_CACHE is None:
        _NC_CACHE = _build_nc()
    return _NC_CACHE


def _prep_in_maps(x, freqs_cos, freqs_sin, wqkv, wo):
    xT = x.reshape(S, DIM).T                                   # [DIM, S]
    # xt[sb, xs, p, n, s] = xT[128*(XSUB*xs+n)+p, 128*sb+s]
    xt = np.ascontiguousarray(
        xT.reshape(NXS, XSUB, 128, NS, 128).transpose(3, 0, 2, 1, 4)
    ).astype(np.float16)
    cos5 = np.ascontiguousarray(np.tile(freqs_cos, (1, 5)))    # [S, 320]
    sin5 = np.ascontiguousarray(np.tile(freqs_sin, (1, 5)))

    # causal masks for the 4 diagonal 128-blocks of a 512-wide s-tile
    tl = np.arange(128)[:, None]
    sl = np.arange(STILE)[None, :]
    cm = np.zeros((128, 4, STILE), np.float32)
    for k in range(4):
        cm[:, k, :] = np.where(sl >= 128 * k + tl, 0.0, MASK_NEG)
    cm = np.ascontiguousarray(cm.reshape(128, 4 * STILE))

    in_maps = []
    for i in range(N_CORES):
        wq = wqkv[QH * HD * i: QH * HD * (i + 1)]               # [512, DIM]
        wk = wqkv[N_HEADS * HD + HD * i: N_HEADS * HD + HD * (i + 1)]
        wv = wqkv[N_HEADS * HD + N_KV_HEADS * HD + HD * i:
                  N_HEADS * HD + N_KV_HEADS * HD + HD * (i + 1)]
        wT = np.concatenate([wq, wk, wv], axis=0).T             # [DIM, 768]
        # wt[p, db, o] = wT[128*db+p, o]
        wt = np.ascontiguousarray(
            wT.reshape(ND, 128, OC).transpose(1, 0, 2)).astype(np.float16)
        woT = wo[:, QH * HD * i: QH * HD * (i + 1)].T           # [512, DIM]
        # wot[p, dc, ob, j] = woT[128*ob+p, 512*dc+j]
        wot = np.ascontiguousarray(
            woT.reshape(QH, 128, NDC, 512).transpose(1, 2, 0, 3)
        ).astype(np.float16)
        in_maps.append({
            "xt": xt, "wt": wt, "wot": wot,
            "cos5": cos5, "sin5": sin5, "cmask": cm,
        })
    return in_maps


def kernel(x, freqs_cos, freqs_sin, mask, wqkv, wo, _want_trace=False):
    x = np.asarray(x, np.float32)
    freqs_cos = np.asarray(freqs_cos, np.float32)
    freqs_sin = np.asarray(freqs_sin, np.float32)
    wqkv = np.asarray(wqkv, np.float32)
    wo = np.asarray(wo, np.float32)

    nc = _get_nc()
    in_maps = _prep_in_maps(x, freqs_cos, freqs_sin, wqkv, wo)
    res = run_bass_kernel_spmd(
        nc, in_maps, core_ids=list(range(N_CORES)), trace=_want_trace,
    )
    out = np.zeros((S, DIM), np.float64)
    for r in res.results:
        out += r["y"].astype(np.float64)
    if _want_trace:
        kernel._last_results = res
    return out.astype(np.float32).reshape(B, S, DIM)
